# revision 60
# baseline (speedup 1.0000x reference)
"""MultiHeadLatentAttention Trainium2 kernel (8 NeuronCores, SPMD).

Sharding: core c -> (batch b = c // 4, latent group g = c % 4).
Each core owns query heads [4g, 4g+4) and latent head g for its batch:
  - q projection for its 4 heads (1/sqrt(HD) folded into the weights)
  - k, v via HOST-MERGED latent weights: k = x @ (kr_w@kl_w)^T + (kr_w@kl_b
    + kr_b) -- mathematically identical to the two-stage latent form but
    4.25x fewer FLOPs since head_dim(128) << latent_dim(512)
  - causal attention for 4 heads in transposed layout (scoresT[s_k, s_q]):
    exp on ScalarE straight out of PSUM (no max subtraction -- scores are
    O(1) by construction), structural causality (diagonal blocks masked by
    a 0/1 tril multiply on probs; masked regions live in persistent
    pre-zeroed tiles), softmax denominators accumulated on DVE in bf16 and
    finished by a SINGLE all-ones matmul per unit, normalization by DVE
    fast-reciprocal multiply
  - partial o_proj (its 512 input dims -> full 2048 output dims), bf16 out
Host sums the 4 partial o_proj outputs per batch (fp32) and adds o_b.

All matmuls run in bf16 with fp32 PSUM accumulation. Scheduling keeps the
in-order PE queue fed: each unit's AV matmuls trail its score matmuls by a
5-block window; o_proj m-blocks of chunk n-1 interleave into chunk n's
attention as PE bubble-fillers (the exp chain on ScalarE is slower per
block than scores+AV on the PE); the denominator matmul + reciprocal +
normalize tails of chunk n are deferred into chunk n+1's projections
(across body boundaries in repeat builds) so the PE never waits on DVE.

HW-measured pitfalls baked into this structure (cost model doesn't see
them): matmuls writing at a nonzero column offset into a PSUM tile
wreck PE pipelining (~2x); PE-mode transposes in the matmul stream cost
~15us/body; each dma_start occupies a shared HWDGE slot ~0.6us, so x
loads are 4 big strided DMAs per chunk; DMAs with wait conditions must
never sit on the ACT HWDGE ring or they stall exp activations.
"""

import math

import numpy as np
import ml_dtypes

B, S, H = 2, 2048, 2048
NH, HD = 16, 128
NLH, LD = 4, 512
HPC = 4            # query heads per core
NCORES = 8
SQC = 512          # s_q chunk
NQC = S // SQC     # 4 chunks
NKT = H // 128     # 16 contraction tiles for the projections
NSB = S // 128     # 16 s_k blocks
BF16 = ml_dtypes.bfloat16

_CACHE = {}


def _build_program(repeat=1):
    import concourse.bacc as bacc
    import concourse.bass as bass
    import concourse.tile as tile
    from concourse import mybir
    from contextlib import ExitStack

    dt = mybir.dt
    AF = mybir.ActivationFunctionType

    nc = bacc.Bacc("TRN2", target_bir_lowering=False, debug=False,
                   num_devices=NCORES)

    xT = nc.declare_dram_parameter("xT", [H, S], dt.bfloat16, isOutput=False)
    qw = nc.declare_dram_parameter("qwT", [H, HPC * HD], dt.bfloat16, isOutput=False)
    kw = nc.declare_dram_parameter("kwT", [H, HD], dt.bfloat16, isOutput=False)
    vw = nc.declare_dram_parameter("vwT", [H, HD], dt.bfloat16, isOutput=False)
    ow = nc.declare_dram_parameter("owT", [HPC * HD, H], dt.bfloat16, isOutput=False)
    qb = nc.declare_dram_parameter("qb", [HPC * HD], dt.float32, isOutput=False)
    kb = nc.declare_dram_parameter("kb", [HD], dt.float32, isOutput=False)
    vb = nc.declare_dram_parameter("vb", [HD], dt.float32, isOutput=False)
    tri = nc.declare_dram_parameter("tri", [128, 128], dt.bfloat16, isOutput=False)
    outp = nc.declare_dram_parameter("out", [H, S], dt.bfloat16, isOutput=True)

    with tile.TileContext(nc) as tc, ExitStack() as ctx:
        const = ctx.enter_context(tc.tile_pool(name="const", bufs=1))
        xpool = ctx.enter_context(tc.tile_pool(name="xpool", bufs=3))
        probs_pool = ctx.enter_context(tc.tile_pool(name="probs", bufs=18))
        attn_pool = ctx.enter_context(tc.tile_pool(name="attn", bufs=8))
        small = ctx.enter_context(tc.tile_pool(name="small", bufs=8))
        psum = ctx.enter_context(tc.tile_pool(name="psum", bufs=8, space="PSUM"))

        # ---------------- constants / weights ----------------
        qw_sb = const.tile([128, NKT, HPC * HD], dt.bfloat16, tag="qw")
        kw_sb = const.tile([128, NKT, HD], dt.bfloat16, tag="kw")
        vw_sb = const.tile([128, NKT, HD], dt.bfloat16, tag="vw")
        # DMA consolidation: the HWDGE descriptor generator is a shared
        # single-slot resource (~0.6us per dma_start regardless of size), so
        # weights load as ONE dma each and x chunks as 4 quarter-dmas.
        # Startup order: x quarters 0-1 on sync; kw, x quarters 2-3, vw,
        # constants, qw on the ACT ring (no wait conditions there ever, so
        # they cannot stall exp activations).
        def dma_x_chunk(n, engines):
            xc = xpool.tile([128, NKT, SQC], dt.bfloat16, tag="xc", name="xc")
            for q in range(4):
                engines[q].dma_start(
                    out=xc[:, 4 * q:4 * (q + 1), :],
                    in_=xT.ap()[512 * q:512 * (q + 1), SQC * n:SQC * (n + 1)]
                    .rearrange("(k p) m -> p k m", p=128))
            return [xc[:, k, :] for k in range(NKT)]

        for k4 in range(4):
            nc.scalar.dma_start(
                out=kw_sb[:, 4 * k4:4 * (k4 + 1), :],
                in_=kw.ap()[512 * k4:512 * (k4 + 1), :]
                .rearrange("(k p) m -> p k m", p=128))
        xs0 = dma_x_chunk(0, [nc.sync, nc.sync, nc.scalar, nc.scalar])
        for k8 in range(2):
            nc.scalar.dma_start(
                out=vw_sb[:, 8 * k8:8 * (k8 + 1), :],
                in_=vw.ap()[1024 * k8:1024 * (k8 + 1), :]
                .rearrange("(k p) m -> p k m", p=128))
        qb_sb = const.tile([128, HPC], dt.float32, tag="qb")
        nc.scalar.dma_start(out=qb_sb, in_=qb.ap().rearrange("(m p) -> p m", p=128))
        kb_sb = const.tile([128, 1], dt.float32, tag="kb")
        nc.scalar.dma_start(out=kb_sb, in_=kb.ap().rearrange("(m p) -> p m", p=128))
        vb_row = const.tile([1, HD], dt.float32, tag="vbr")
        nc.scalar.dma_start(out=vb_row,
                            in_=vb.ap().rearrange("(p m) -> p m", p=1))
        tri_sb = const.tile([128, 128], dt.bfloat16, tag="tri")
        nc.scalar.dma_start(out=tri_sb, in_=tri.ap())
        ones_sb = const.tile([128, 128], dt.bfloat16, tag="ones")
        nc.vector.memset(ones_sb, 1.0)
        ones1f = const.tile([1, 128], dt.float32, tag="ones1f")
        nc.vector.memset(ones1f, 1.0)
        vb_bc = const.tile([128, HD], dt.float32, tag="vbb")
        for k4 in range(4):
            nc.scalar.dma_start(
                out=qw_sb[:, 4 * k4:4 * (k4 + 1), :],
                in_=qw.ap()[512 * k4:512 * (k4 + 1), :]
                .rearrange("(k p) m -> p k m", p=128))

        # persistent activations
        qT_sb = [const.tile([128, S], dt.bfloat16, tag=f"qT{h}", name=f"qT{h}")
                 for h in range(HPC)]
        kT_sb = const.tile([128, S], dt.bfloat16, tag="kT")
        v_sb = const.tile([128, NSB, HD], dt.bfloat16, tag="v")

        # dedicated diagonal-block prob tiles (two parity sets to relax WAR
        # serialization between consecutive units): the masked column range
        # [0, 128*d) is zeroed ONCE here and never overwritten -- exp and the
        # tril mask only touch [128*d:] -- killing ~48 DVE memsets per body.
        dpt = {}
        for par in range(2):
            for d in range(1, 4):
                t = const.tile([128, SQC], dt.bfloat16, tag=f"dpt{par}{d}",
                               name=f"dpt{par}{d}")
                nc.vector.memset(t[:, :128 * d], 0.0)
                dpt[(par, d)] = t

        # o_proj and unit tails (ones-matmul -> recip -> at-mul) for chunk n
        # are deferred until inside chunk n+1's projections (crossing body
        # boundaries in repeat builds): the PE crunches independent phase-1
        # matmuls while the DVE prob-sum chains drain, instead of stalling
        # the in-order PE queue at the denominator matmul / o_proj.
        pending_tails = []
        pending_oproj = [None]
        ow_sb = [None]

        def _emit_body(first):
            # ---------------- phase 1: projections ----------------
            for n in range(NQC):
                if first and n == 0:
                    xs = xs0
                else:
                    xs = dma_x_chunk(
                        n, [nc.sync, nc.sync, nc.gpsimd, nc.gpsimd])

                # kT[:, n-chunk] = (kr_w @ kl_w) @ x^T  (host-merged weight)
                ps = psum.tile([128, SQC], dt.float32, tag="bank")
                for k in range(NKT):
                    nc.tensor.matmul(ps, lhsT=kw_sb[:, k, :], rhs=xs[k],
                                     start=(k == 0), stop=(k == NKT - 1))
                nc.scalar.activation(out=kT_sb[:, SQC * n:SQC * (n + 1)], in_=ps,
                                     func=AF.Identity, bias=kb_sb[:, 0:1])
                if pending_tails:
                    pending_tails.pop(0)()

                if first and n == 0:
                    ps_vb = psum.tile([128, HD], dt.float32, tag="bank",
                                      name="ps_vb")
                    nc.tensor.matmul(ps_vb, lhsT=ones1f, rhs=vb_row,
                                     start=True, stop=True)
                    nc.vector.tensor_copy(out=vb_bc, in_=ps_vb)

                # v natural [s, hd] = x @ (vr_w @ vl_w)^T  (host-merged weight)
                for jj in range(4):
                    ps = psum.tile([128, SQC], dt.float32, tag="bank")
                    for k in range(NKT):
                        nc.tensor.matmul(ps[:, :HD],
                                         lhsT=xs[k][:, 128 * jj:128 * (jj + 1)],
                                         rhs=vw_sb[:, k, :],
                                         start=(k == 0), stop=(k == NKT - 1))
                    nc.vector.tensor_add(out=v_sb[:, 4 * n + jj, :], in0=ps[:, :HD],
                                         in1=vb_bc)
                    if pending_tails:
                        pending_tails.pop(0)()

                for h in range(HPC):
                    ps = psum.tile([128, SQC], dt.float32, tag="bank")
                    for k in range(NKT):
                        nc.tensor.matmul(ps, lhsT=qw_sb[:, k, 128 * h:128 * (h + 1)],
                                         rhs=xs[k], start=(k == 0), stop=(k == NKT - 1))
                    nc.scalar.activation(out=qT_sb[h][:, SQC * n:SQC * (n + 1)], in_=ps,
                                         func=AF.Identity, bias=qb_sb[:, h:h + 1])

                if first and n == 1:
                    # o_proj weights: first consumed in o_proj of iq=0, which
                    # runs after this chunk's projections; queued behind the
                    # n=1 x tiles so it doesn't delay them.
                    ow_sb[0] = const.tile([128, 4, H], dt.bfloat16, tag="ow",
                                          name="ow_sb")
                    for k in range(4):
                        nc.scalar.dma_start(
                            out=ow_sb[0][:, k:k + 1, :],
                            in_=ow.ap()[128 * k:128 * (k + 1), :]
                            .rearrange("(k p) m -> p k m", p=128))

                # o_proj m-blocks of the previous chunk interleave into this
                # chunk's attention as PE bubble-fillers: the exp chain on
                # ScalarE (~612ns/block) outpaces the PE's scores+AV
                # (~426ns/block) in big units, so the in-order PE queue gets
                # filler o_proj matmuls between AV matmuls.
                J = 4 * n + 4
                fill_state = {
                    "fillers": pending_oproj[0] or [],
                    "navs": 0,
                    "rate": max(1, (HPC * J) // NSB),
                }
                pending_oproj[0] = None

                # ---- attention + o_proj for iq = n (kT/v/qT chunks 0..n ready)
                # Each unit's tail AV/sums + normalize are deferred until after
                # the NEXT unit's scores, so the PE never waits on the
                # exp->mask chain of the current unit.
                iq = n
                attn_tiles = []
                prev_tail = None

                def make_unit(h):
                    J = 4 * iq + 4
                    state = {"av": None, "acc": None, "pt0": None}

                    def emit_av(j, pt):
                        if state["av"] is None:
                            state["av"] = psum.tile([128, SQC], dt.float32,
                                                    tag="bank", name="ps_av")
                        nc.tensor.matmul(state["av"], lhsT=v_sb[:, j, :], rhs=pt,
                                         start=(j == 0), stop=(j == J - 1))
                        fill_state["navs"] += 1
                        if (fill_state["fillers"]
                                and fill_state["navs"] % fill_state["rate"] == 0):
                            fill_state["fillers"].pop(0)()
                        # softmax denominator: accumulate probs on DVE (bf16),
                        # finished by a single ones-matmul in the tail -- saves
                        # J-1 PE matmuls per unit vs matmul-accumulated sums.
                        if state["acc"] is None:
                            if state["pt0"] is None:
                                state["pt0"] = pt
                            else:
                                state["acc"] = small.tile(
                                    [128, SQC], dt.bfloat16, tag="acc",
                                    name="acc")
                                nc.vector.tensor_add(out=state["acc"],
                                                     in0=state["pt0"], in1=pt)
                                state["pt0"] = None
                        else:
                            nc.vector.tensor_add(out=state["acc"],
                                                 in0=state["acc"], in1=pt)

                    pending = []

                    def emit_scores():
                        nonlocal pending
                        for j in range(J):
                            ps_s = psum.tile([128, SQC], dt.float32, tag="bank",
                                             name="ps_s")
                            nc.tensor.matmul(ps_s,
                                             lhsT=kT_sb[:, 128 * j:128 * (j + 1)],
                                             rhs=qT_sb[h][:, SQC * iq:SQC * (iq + 1)],
                                             start=True, stop=True)
                            d = j - 4 * iq
                            if d <= 0:
                                pt = probs_pool.tile([128, SQC], dt.bfloat16,
                                                     tag="pt", name="pt")
                            else:
                                pt = dpt[(h % 2, d)]
                            if d < 0:
                                nc.scalar.activation(out=pt, in_=ps_s, func=AF.Exp)
                            else:
                                nc.scalar.activation(out=pt[:, 128 * d:],
                                                     in_=ps_s[:, 128 * d:],
                                                     func=AF.Exp)
                                nc.vector.tensor_mul(
                                    out=pt[:, 128 * d:128 * (d + 1)],
                                    in0=pt[:, 128 * d:128 * (d + 1)], in1=tri_sb)
                            pending.append((j, pt))
                            if len(pending) > 6:
                                emit_av(*pending.pop(0))

                    def emit_av_flush():
                        for p in pending:
                            emit_av(*p)

                    def emit_fin(slot=h):
                        ps_sum = psum.tile([128, SQC], dt.float32, tag="bank",
                                           name="ps_sum")
                        nc.tensor.matmul(ps_sum, lhsT=ones_sb, rhs=state["acc"],
                                         start=True, stop=True)
                        recip = small.tile([128, SQC], dt.float32, tag="recip",
                                           name="recip")
                        nc.vector.reciprocal_approx_fast(out=recip, in_=ps_sum)
                        at = attn_pool.tile([128, SQC], dt.bfloat16, tag="at",
                                            name="at")
                        nc.vector.tensor_mul(out=at, in0=state["av"], in1=recip)
                        attn_tiles[slot] = at

                    return emit_scores, emit_av_flush, emit_fin

                for h in range(HPC):
                    attn_tiles.append(None)
                    emit_scores, emit_av_flush, emit_fin = make_unit(h)
                    emit_scores()
                    if prev_tail is not None:
                        prev_tail()
                    prev_tail = emit_av_flush
                    pending_tails.append(emit_fin)
                prev_tail()
                for f in fill_state["fillers"]:
                    f()
                fill_state["fillers"] = []

                def make_oblock(m, iq=iq, attn_tiles=attn_tiles):
                    def emit_oblock():
                        ps_o = psum.tile([128, SQC], dt.float32, tag="bank")
                        for h in range(HPC):
                            nc.tensor.matmul(
                                ps_o, lhsT=ow_sb[0][:, h, 128 * m:128 * (m + 1)],
                                rhs=attn_tiles[h], start=(h == 0), stop=(h == 3))
                        o_sb = small.tile([128, SQC], dt.bfloat16, tag="osb")
                        nc.vector.tensor_copy(out=o_sb, in_=ps_o)
                        nc.sync.dma_start(
                            out=outp.ap()[128 * m:128 * (m + 1),
                                          SQC * iq:SQC * (iq + 1)],
                            in_=o_sb)
                    return emit_oblock
                pending_oproj[0] = [make_oblock(m) for m in range(NSB)]

        for _rep in range(repeat):
            _emit_body(first=(_rep == 0))
        while pending_tails:
            pending_tails.pop(0)()
        for f in pending_oproj[0]:
            f()

    nc.compile()
    return nc


def _get_nc(repeat=1):
    key = f"nc{repeat}"
    if key not in _CACHE:
        _CACHE[key] = _build_program(repeat)
    return _CACHE[key]


def _make_in_maps(hidden_states, attention_mask, q_w, q_b, kl_w, kl_b, vl_w, vl_b,
                  kr_w, kr_b, vr_w, vr_b, o_w):
    scale = 1.0 / math.sqrt(HD)
    tri01 = (np.asarray(attention_mask[0, 0, :128, :128]) == 0).T.astype(BF16)
    kr_f = np.asarray(kr_w, np.float32)
    vr_f = np.asarray(vr_w, np.float32)
    in_maps = []
    for c in range(NCORES):
        b, g = divmod(c, NLH)
        sl = slice(LD * g, LD * (g + 1))
        xTc = np.ascontiguousarray(np.asarray(hidden_states[b], np.float32).T
                                   ).astype(BF16)
        # merged latent->head weights: k = x @ (kr_w @ kl_w)^T + (kr_w@kl_b + kr_b)
        kw_eff = kr_f @ np.asarray(kl_w[sl], np.float32)
        vw_eff = vr_f @ np.asarray(vl_w[sl], np.float32)
        kb_eff = kr_f @ np.asarray(kl_b[sl], np.float32) + np.asarray(kr_b, np.float32)
        vb_eff = vr_f @ np.asarray(vl_b[sl], np.float32) + np.asarray(vr_b, np.float32)
        in_maps.append({
            "xT": xTc,
            "qwT": np.ascontiguousarray(
                (np.asarray(q_w[sl], np.float32) * scale).T).astype(BF16),
            "kwT": np.ascontiguousarray(kw_eff.T).astype(BF16),
            "vwT": np.ascontiguousarray(vw_eff.T).astype(BF16),
            "owT": np.ascontiguousarray(np.asarray(o_w, np.float32)[:, sl].T
                                        ).astype(BF16),
            "qb": (np.asarray(q_b[sl], np.float32) * scale),
            "kb": kb_eff,
            "vb": vb_eff,
            "tri": tri01,
        })
    return in_maps


def _gather(results, o_b):
    o_b = np.asarray(o_b, np.float32)
    outs = []
    for b in range(B):
        acc = np.zeros((H, S), np.float32)
        for g in range(NLH):
            acc += results[b * NLH + g]["out"].astype(np.float32)
        outs.append(acc.T + o_b[None, :])
    return np.stack(outs).astype(np.float32)


def kernel(hidden_states, position_ids, attention_mask, q_w, q_b, kl_w, kl_b,
           vl_w, vl_b, kr_w, kr_b, vr_w, vr_b, o_w, o_b):
    from concourse.bass_utils import run_bass_kernel_spmd

    nc = _get_nc()
    in_maps = _make_in_maps(hidden_states, attention_mask, q_w, q_b, kl_w, kl_b,
                            vl_w, vl_b, kr_w, kr_b, vr_w, vr_b, o_w)
    res = run_bass_kernel_spmd(nc, in_maps, core_ids=list(range(NCORES)))
    return _gather(res.results, o_b)

